# revision 16
# baseline (speedup 1.0000x reference)
"""LookupConv2d kernel for Trainium2 (8 NeuronCores, data-parallel over batch).

Computation: weight[o] = sum_s coeff[o,s] * dictionary[idx[o,s]]  (tiny, host)
             out = conv2d(x, weight, stride 1, pad 1)             (device)

v2: Winograd F(2,3) along H in bf16 — 1.5x fewer PE MACs than direct conv.

  out[o, 2p+0, x] = m0 + m1 + m2          m_t[o,p,x] = sum_c sum_kx
  out[o, 2p+1, x] = m1 - m2 - m3                Gw[c,t,kx,o] * XT[c,t,p,x+kx]

  XT (host):  Bt rows of padded x:  [d0-d2, d1+d2, d2-d1, d1-d3],
              d_k = xpad[:, 2p+k, :]  -> [C, img, sb=4, t=4, p=7, 58] bf16
              (row-pair superblocks sb-major so each is one contiguous DMA)
  Gw (host):  G @ w over ky, G = [[1,0,0],[.5,.5,.5],[.5,-.5,.5],[0,0,1]]
              -> lhsT [C=128, co=2, t=4, kx=3, o=128] bf16

Device per core (4 images):
  - Per (img, co-half, superblock of 7 row-pairs): 12 matmuls (4t x 3kx)
    of N=392 accumulate m_t over kx into 4 PSUM banks (x2 sets for double
    buffering = all 8 banks).
  - ACT copies m0,m1,m2 PSUM->SBUF as bf16 (ACT is otherwise idle; keeps
    DVE off the 1x fp32 tensor_tensor path).
  - DVE (bf16 2x): t0=s0+s1; even=t0+s2; t1=s1-s2; odd=t1-m3(psum).
    Even/odd row planes stored separately; host interleaves (free).
  - All DMAs ride one HW queue in issue order, so issue them in consumption
    order: img0/sb0, co0 weights, img0/sb1-3, co1 weights, img1, imgs2+3.
  - PE warmup: dummy matmuls on a memset tile bridge the DMA wait so the
    HAM clock gate reaches 2.4 GHz before real matmuls start; absorber
    matmuls advance PE's clock past each input DMA so real matmuls carry
    at most one sync wait.
"""

import numpy as np
from contextlib import ExitStack

import concourse.bass as bass
import concourse.bacc as bacc
import concourse.tile as tile
from concourse import mybir
from concourse.bass_utils import run_bass_kernel_spmd

N_CORES = 8
B, CIN, H, W = 32, 128, 56, 56
COUT = 256
KK = 3
HP, WP = H + 2, W + 2  # padded 58, 58
BPC = B // N_CORES  # 4 images per core
NPAIR = H // 2  # 28 output row pairs
NT = 4  # winograd points
BF = mybir.dt.bfloat16
F32 = mybir.dt.float32

# 4 superblocks of 7 row-pairs each (uniform -> contiguous sb-major DMA)
NSB = 4
PSB = 7  # pairs per superblock; matmul N = 7*56 = 392
N_WARMUP = 56  # dummy matmuls (N=56, ~47ns cold) bridging preamble->data

_CACHE: dict = {}


def _build_program():
    nc = bacc.Bacc("TRN2", target_bir_lowering=False, debug=False)
    xs = nc.dram_tensor("xs", [CIN, BPC, NSB, NT, PSB, WP], BF, kind="ExternalInput")
    wt = nc.dram_tensor("wt", [CIN, 2, NT, KK, 128], BF, kind="ExternalInput")
    out = nc.dram_tensor(
        "out", [CIN, BPC, 2, 2, NSB, PSB, W], BF, kind="ExternalOutput"
    )

    with tile.TileContext(nc) as tc, ExitStack() as ctx:
        xpool = ctx.enter_context(tc.tile_pool(name="x", bufs=1))
        wpool = ctx.enter_context(tc.tile_pool(name="w", bufs=1))
        opool = ctx.enter_context(tc.tile_pool(name="o", bufs=1))
        ppool = ctx.enter_context(tc.tile_pool(name="p", bufs=1, space="PSUM"))
        spool = ctx.enter_context(tc.tile_pool(name="s", bufs=4))
        tpool = ctx.enter_context(tc.tile_pool(name="t", bufs=2))

        # 2 sets x 4 winograd points of PSUM accumulators = all 8 banks
        pt = [
            [
                ppool.tile([128, PSB, W], F32, name=f"ps{s}t{t}", tag=f"ps{s}t{t}")
                for t in range(NT)
            ]
            for s in range(2)
        ]
        scr = pt[1][0][:, 0, 0:2]  # absorber target (set B unused at start)

        wu = wpool.tile([128, 128], BF, tag="wu")
        nc.gpsimd.memset(wu[:], 0.0)
        for _ in range(N_WARMUP):
            nc.tensor.matmul(
                pt[1][0][:, 0, :], wu[:, 0:128], wu[:, 0:W], start=True, stop=True
            )

        def absorb(rhs2):
            nc.tensor.matmul(scr, wu[:, 0:128], rhs2, start=True, stop=True)

        # Input DMAs: each issuing engine triggers its own HW DMA ring, so
        # spreading issues across engines parallelizes the transfers.
        xt0 = xpool.tile([CIN, 1, NSB, NT, PSB, WP], BF, tag="x0")
        nc.sync.dma_start(xt0[:, :, 0], xs[:, 0:1, 0])
        w_all = wpool.tile([CIN, 2, NT, KK, 128], BF)
        nc.sync.dma_start(w_all[:, 0], wt[:, 0])
        nc.gpsimd.dma_start(xt0[:, :, 1:NSB], xs[:, 0:1, 1:NSB])
        nc.scalar.dma_start(w_all[:, 1], wt[:, 1])
        xt1 = xpool.tile([CIN, 1, NSB, NT, PSB, WP], BF, tag="x1")
        nc.gpsimd.dma_start(xt1[:], xs[:, 1:2])
        xt23 = xpool.tile([CIN, 2, NSB, NT, PSB, WP], BF, tag="x23")
        nc.sync.dma_start(xt23[:], xs[:, 2:4])
        xv = [(xt0, 0), (xt1, 0), (xt23, 0), (xt23, 1)]

        absorb(xt0[:, 0, 0, 0, 0, 0:2])  # img0 sb0
        absorb(w_all[:, 0, 0, 0, 0:2])  # co0 weights

        sb = 0  # global superblock counter for PSUM set alternation
        ot01 = None
        for img in range(BPC):
            xt, j = xv[img]
            if img == 1:
                absorb(xt1[:, 0, 0, 0, 0, 0:2])
            elif img == 2:  # img3 shares img2's DMA; PE already observed it
                absorb(xt23[:, 0, 0, 0, 0, 0:2])
            if img == 0:
                ot01 = opool.tile([128, 2, 2, 2, NSB, PSB, W], BF, tag="o01")
                ot = ot01[:, 0]
            elif img == 1:
                ot = ot01[:, 1]
            else:
                ot = opool.tile([128, 2, 2, NSB, PSB, W], BF, tag=f"o{img}")
            for co in range(2):
                if img == 0 and co == 1:
                    absorb(w_all[:, 1, 0, 0, 0:2])  # co1 weights
                for sbi in range(NSB):
                    if img == 0 and co == 0 and sbi == 1:
                        absorb(xt0[:, 0, 1, 0, 0, 0:2])  # img0 sb1-3
                    s = sb % 2
                    sb += 1
                    # group order t1,t2,t3,t0: the only work left after the
                    # last (t0) matmul group is one DVE op -> minimal tail
                    for t in (1, 2, 3, 0):
                        for kx in range(KK):
                            nc.tensor.matmul(
                                pt[s][t][:],
                                w_all[:, co, t, kx, :],
                                xt[:, j, sbi, t, :, kx : kx + W],
                                start=(kx == 0),
                                stop=(kx == KK - 1),
                            )
                    # ACT: bf16 copies of m1,m2,m3 (PSUM -> SBUF)
                    sm = {}
                    for t in (1, 2, 3):
                        sm[t] = spool.tile([128, PSB, W], BF, name=f"sm{t}")
                        nc.scalar.copy(sm[t][:], pt[s][t][:])
                    # DVE: even = (m1+m2)+m0 ; odd = (m1-m2)-m3
                    t0 = tpool.tile([128, PSB, W], BF)
                    nc.vector.tensor_add(t0[:], sm[1][:], sm[2][:])
                    t1 = tpool.tile([128, PSB, W], BF)
                    nc.vector.tensor_sub(t1[:], sm[1][:], sm[2][:])
                    nc.vector.tensor_sub(ot[:, co, 1, sbi], t1[:], sm[3][:])
                    nc.vector.tensor_add(ot[:, co, 0, sbi], t0[:], pt[s][0][:])
                if img == 3 and co == 0:
                    # flush img3/co0 while co1 computes
                    nc.sync.dma_start(out[:, 3, 0], ot[:, 0])
            if img == 1:
                nc.sync.dma_start(out[:, 0:2], ot01[:])
            elif img == 2:
                nc.sync.dma_start(out[:, 2], ot[:])
            elif img == 3:
                nc.sync.dma_start(out[:, 3, 1], ot[:, 1])
    nc.compile()
    return nc


def _get_program():
    if "nc" not in _CACHE:
        _CACHE["nc"] = _build_program()
    return _CACHE["nc"]


def _prepare_inputs(x, dictionary, lookup_coefficients, lookup_indices):
    import ml_dtypes

    bf16 = ml_dtypes.bfloat16
    x = np.asarray(x, dtype=np.float32)
    dictionary = np.asarray(dictionary, dtype=np.float32)
    coeff = np.asarray(lookup_coefficients, dtype=np.float32)
    idx = np.asarray(lookup_indices)

    # Compose per-output-channel filters on host (2.4 MFLOP - negligible).
    atoms = dictionary[idx]  # (Cout, S, Cin, K, K)
    weight = np.einsum("os,osckl->ockl", coeff, atoms)  # (Cout, Cin, 3, 3)
    # Winograd G @ w over ky -> lhsT [c, co, t, kx, o128]
    G = np.array([[1, 0, 0], [0.5, 0.5, 0.5], [0.5, -0.5, 0.5], [0, 0, 1]], np.float32)
    Gw = np.einsum("tk,ockx->ctxo", G, weight)  # (128, 4, 3, 256)
    Gw = Gw.reshape(CIN, NT, KK, 2, 128).transpose(0, 3, 1, 2, 4)
    wt_host = np.ascontiguousarray(Gw).astype(bf16)  # (128, 2, 4, 3, 128)

    # Pad, then Bt row transform: pairs p use padded rows 2p..2p+3.
    x_pad = np.zeros((B, CIN, HP, WP), dtype=np.float32)
    x_pad[:, :, 1 : H + 1, 1 : W + 1] = x
    d = [x_pad[:, :, k : k + 2 * (NPAIR - 1) + 1 : 2, :] for k in range(4)]
    XT = np.stack(
        [d[0] - d[2], d[1] + d[2], d[2] - d[1], d[1] - d[3]], axis=2
    )  # (B, C, 4t, 28p, 58)
    # regroup pairs into superblocks: (B, C, t, sb, p7, w) -> (B, C, sb, t, p7, w)
    XT = XT.reshape(B, CIN, NT, NSB, PSB, WP).transpose(0, 1, 3, 2, 4, 5)
    XT = np.ascontiguousarray(XT).astype(bf16)

    in_maps = []
    for c in range(N_CORES):
        xs_core = np.ascontiguousarray(
            XT[c * BPC : (c + 1) * BPC].transpose(1, 0, 2, 3, 4, 5)
        )  # (C, img, sb, t, p7, 58)
        in_maps.append({"xs": xs_core, "wt": wt_host})
    return in_maps


def _ensure_ntff_hook() -> bool:
    """Register the axon NTFF profile hook (missing antenv.axon_hooks shim).

    Only needed for trace=True runs; grading path (trace=False) never calls it.
    """
    import sys
    import types
    import contextlib
    import ctypes

    try:
        import antenv.axon_hooks as m  # noqa: F401
        if m.get_axon_ntff_profile_hook() is not None:
            return True
    except ImportError:
        m = types.ModuleType("antenv.axon_hooks")
        _h = {"hook": None}
        m.set_axon_ntff_profile_hook = lambda h: _h.__setitem__("hook", h)
        m.get_axon_ntff_profile_hook = lambda: _h["hook"]
        sys.modules["antenv.axon_hooks"] = m
        try:
            import antenv
            antenv.axon_hooks = m
        except ImportError:
            pass

    so_path = "/opt/axon/libaxon_pjrt.so"
    try:
        lib = ctypes.CDLL(so_path)
    except OSError:
        return False
    if not hasattr(lib, "axon_start_nrt_profile"):
        return False
    lib.axon_start_nrt_profile.argtypes = [
        ctypes.POINTER(ctypes.c_int64),
        ctypes.c_size_t,
    ]
    lib.axon_start_nrt_profile.restype = ctypes.c_int64
    lib.axon_stop_nrt_profile.argtypes = [ctypes.c_char_p]
    lib.axon_stop_nrt_profile.restype = ctypes.c_int64

    @contextlib.contextmanager
    def _hook(output_dir, device_ids):
        import jax

        jax.devices()
        if device_ids:
            ids = (ctypes.c_int64 * len(device_ids))(*device_ids)
            rc = lib.axon_start_nrt_profile(ids, len(device_ids))
        else:
            rc = lib.axon_start_nrt_profile(None, 0)
        if rc != 0:
            raise RuntimeError(f"axon_start_nrt_profile rc={rc}")
        try:
            yield
        finally:
            n = lib.axon_stop_nrt_profile(str(output_dir).encode())
            if n < 0:
                raise RuntimeError(f"axon_stop_nrt_profile rc={n}")
            print(f"profile: {n} file(s) written to {output_dir}", file=sys.stderr)

    m.set_axon_ntff_profile_hook(_hook)
    return True


def _run(inputs: dict, trace: bool = False):
    if trace:
        trace = _ensure_ntff_hook()
    nc = _get_program()
    in_maps = _prepare_inputs(**inputs)
    res = run_bass_kernel_spmd(nc, in_maps, list(range(N_CORES)), trace=trace)
    out = np.empty((B, COUT, H, W), dtype=np.float32)
    for c in range(N_CORES):
        # device layout: (p, img, co, eo, sb, p7, x)
        #   -> (img, co*128+p, 2*(sb*7+p7)+eo, x)
        arr = np.asarray(res.results[c]["out"]).reshape(128, BPC, 2, 2, NPAIR, W)
        out[c * BPC : (c + 1) * BPC] = (
            arr.transpose(1, 2, 0, 4, 3, 5)
            .reshape(BPC, COUT, H, W)
            .astype(np.float32)
        )
    return out, res


def kernel(**inputs) -> np.ndarray:
    out, _ = _run(inputs, trace=False)
    return out


# revision 17
# speedup vs baseline: 1.3539x; 1.3539x over previous
"""LookupConv2d kernel for Trainium2 (8 NeuronCores, data-parallel over batch).

Computation: weight[o] = sum_s coeff[o,s] * dictionary[idx[o,s]]  (tiny, host)
             out = conv2d(x, weight, stride 1, pad 1)             (device)

v2: Winograd F(2,3) along H in bf16 — 1.5x fewer PE MACs than direct conv.

  out[o, 2p+0, x] = m0 + m1 + m2          m_t[o,p,x] = sum_c sum_kx
  out[o, 2p+1, x] = m1 - m2 - m3                Gw[c,t,kx,o] * XT[c,t,p,x+kx]

  XT (host):  Bt rows of padded x:  [d0-d2, d1+d2, d2-d1, d1-d3],
              d_k = xpad[:, 2p+k, :]  -> [C, img, sb=4, t=4, p=7, 58] bf16
              (row-pair superblocks sb-major so each is one contiguous DMA)
  Gw (host):  G @ w over ky, G = [[1,0,0],[.5,.5,.5],[.5,-.5,.5],[0,0,1]]
              -> lhsT [C=128, co=2, t=4, kx=3, o=128] bf16

Device per core (4 images):
  - Per (img, co-half, superblock of 7 row-pairs): 12 matmuls (4t x 3kx)
    of N=392 accumulate m_t over kx into 4 PSUM banks (x2 sets for double
    buffering = all 8 banks).
  - ACT copies m0,m1,m2 PSUM->SBUF as bf16 (ACT is otherwise idle; keeps
    DVE off the 1x fp32 tensor_tensor path).
  - DVE (bf16 2x): t0=s0+s1; even=t0+s2; t1=s1-s2; odd=t1-m3(psum).
    Even/odd row planes stored separately; host interleaves (free).
  - All DMAs ride one HW queue in issue order, so issue them in consumption
    order: img0/sb0, co0 weights, img0/sb1-3, co1 weights, img1, imgs2+3.
  - PE warmup: dummy matmuls on a memset tile bridge the DMA wait so the
    HAM clock gate reaches 2.4 GHz before real matmuls start; absorber
    matmuls advance PE's clock past each input DMA so real matmuls carry
    at most one sync wait.
"""

import numpy as np
from contextlib import ExitStack

import concourse.bass as bass
import concourse.bacc as bacc
import concourse.tile as tile
from concourse import mybir
from concourse.bass_utils import run_bass_kernel_spmd

N_CORES = 8
B, CIN, H, W = 32, 128, 56, 56
COUT = 256
KK = 3
HP, WP = H + 2, W + 2  # padded 58, 58
BPC = B // N_CORES  # 4 images per core
NPAIR = H // 2  # 28 output row pairs
NT = 4  # winograd points
BF = mybir.dt.bfloat16
F32 = mybir.dt.float32

# 4 superblocks of 7 row-pairs each (uniform -> contiguous sb-major DMA)
NSB = 4
PSB = 7  # pairs per superblock; matmul N = 7*56 = 392
N_WARMUP = 56  # dummy matmuls (N=56, ~47ns cold) bridging preamble->data

_CACHE: dict = {}


def _build_program():
    nc = bacc.Bacc("TRN2", target_bir_lowering=False, debug=False)
    xs = nc.dram_tensor("xs", [CIN, BPC, NSB, NT, PSB, WP], BF, kind="ExternalInput")
    wt = nc.dram_tensor("wt", [CIN, 2, NT, KK, 128], BF, kind="ExternalInput")
    out = nc.dram_tensor(
        "out", [CIN, BPC, 2, 2, NSB, PSB, W], BF, kind="ExternalOutput"
    )

    with tile.TileContext(nc) as tc, ExitStack() as ctx:
        xpool = ctx.enter_context(tc.tile_pool(name="x", bufs=1))
        wpool = ctx.enter_context(tc.tile_pool(name="w", bufs=1))
        opool = ctx.enter_context(tc.tile_pool(name="o", bufs=1))
        ppool = ctx.enter_context(tc.tile_pool(name="p", bufs=1, space="PSUM"))
        spool = ctx.enter_context(tc.tile_pool(name="s", bufs=4))
        tpool = ctx.enter_context(tc.tile_pool(name="t", bufs=2))

        # 2 sets x 4 winograd points of PSUM accumulators = all 8 banks
        pt = [
            [
                ppool.tile([128, PSB, W], F32, name=f"ps{s}t{t}", tag=f"ps{s}t{t}")
                for t in range(NT)
            ]
            for s in range(2)
        ]
        scr = pt[1][0][:, 0, 0:2]  # absorber target (set B unused at start)

        wu = wpool.tile([128, 128], BF, tag="wu")
        nc.gpsimd.memset(wu[:], 0.0)
        for _ in range(N_WARMUP):
            nc.tensor.matmul(
                pt[1][0][:, 0, :], wu[:, 0:128], wu[:, 0:W], start=True, stop=True
            )

        def absorb(rhs2):
            nc.tensor.matmul(scr, wu[:, 0:128], rhs2, start=True, stop=True)

        # Input DMAs: each issuing engine triggers its own HW DMA ring, so
        # spreading issues across engines parallelizes the transfers.
        xt0 = xpool.tile([CIN, 1, NSB, NT, PSB, WP], BF, tag="x0")
        nc.sync.dma_start(xt0[:, :, 0], xs[:, 0:1, 0])
        w_all = wpool.tile([CIN, 2, NT, KK, 128], BF)
        nc.sync.dma_start(w_all[:, 0], wt[:, 0])
        nc.sync.dma_start(xt0[:, :, 1:NSB], xs[:, 0:1, 1:NSB])
        nc.sync.dma_start(w_all[:, 1], wt[:, 1])
        xt1 = xpool.tile([CIN, 1, NSB, NT, PSB, WP], BF, tag="x1")
        nc.sync.dma_start(xt1[:], xs[:, 1:2])
        xt23 = xpool.tile([CIN, 2, NSB, NT, PSB, WP], BF, tag="x23")
        nc.sync.dma_start(xt23[:], xs[:, 2:4])
        xv = [(xt0, 0), (xt1, 0), (xt23, 0), (xt23, 1)]

        absorb(xt0[:, 0, 0, 0, 0, 0:2])  # img0 sb0
        absorb(w_all[:, 0, 0, 0, 0:2])  # co0 weights

        sb = 0  # global superblock counter for PSUM set alternation
        ot01 = None
        for img in range(BPC):
            xt, j = xv[img]
            if img == 1:
                absorb(xt1[:, 0, 0, 0, 0, 0:2])
            elif img == 2:  # img3 shares img2's DMA; PE already observed it
                absorb(xt23[:, 0, 0, 0, 0, 0:2])
            if img == 0:
                ot01 = opool.tile([128, 2, 2, 2, NSB, PSB, W], BF, tag="o01")
                ot = ot01[:, 0]
            elif img == 1:
                ot = ot01[:, 1]
            else:
                ot = opool.tile([128, 2, 2, NSB, PSB, W], BF, tag=f"o{img}")
            for co in range(2):
                if img == 0 and co == 1:
                    absorb(w_all[:, 1, 0, 0, 0:2])  # co1 weights
                for sbi in range(NSB):
                    if img == 0 and co == 0 and sbi == 1:
                        absorb(xt0[:, 0, 1, 0, 0, 0:2])  # img0 sb1-3
                    s = sb % 2
                    sb += 1
                    # group order t1,t2,t3,t0: the only work left after the
                    # last (t0) matmul group is one DVE op -> minimal tail
                    for t in (1, 2, 3, 0):
                        for kx in range(KK):
                            nc.tensor.matmul(
                                pt[s][t][:],
                                w_all[:, co, t, kx, :],
                                xt[:, j, sbi, t, :, kx : kx + W],
                                start=(kx == 0),
                                stop=(kx == KK - 1),
                            )
                    # ACT: bf16 copies of m1,m2,m3 (PSUM -> SBUF)
                    sm = {}
                    for t in (1, 2, 3):
                        sm[t] = spool.tile([128, PSB, W], BF, name=f"sm{t}")
                        nc.scalar.copy(sm[t][:], pt[s][t][:])
                    # DVE: even = (m1+m2)+m0 ; odd = (m1-m2)-m3
                    t0 = tpool.tile([128, PSB, W], BF)
                    nc.vector.tensor_add(t0[:], sm[1][:], sm[2][:])
                    t1 = tpool.tile([128, PSB, W], BF)
                    nc.vector.tensor_sub(t1[:], sm[1][:], sm[2][:])
                    nc.vector.tensor_sub(ot[:, co, 1, sbi], t1[:], sm[3][:])
                    nc.vector.tensor_add(ot[:, co, 0, sbi], t0[:], pt[s][0][:])
                if img == 3 and co == 0:
                    # flush img3/co0 while co1 computes
                    nc.sync.dma_start(out[:, 3, 0], ot[:, 0])
            if img == 1:
                nc.sync.dma_start(out[:, 0:2], ot01[:])
            elif img == 2:
                nc.sync.dma_start(out[:, 2], ot[:])
            elif img == 3:
                nc.sync.dma_start(out[:, 3, 1], ot[:, 1])
    nc.compile()
    return nc


def _get_program():
    if "nc" not in _CACHE:
        _CACHE["nc"] = _build_program()
    return _CACHE["nc"]


def _prepare_inputs(x, dictionary, lookup_coefficients, lookup_indices):
    import ml_dtypes

    bf16 = ml_dtypes.bfloat16
    x = np.asarray(x, dtype=np.float32)
    dictionary = np.asarray(dictionary, dtype=np.float32)
    coeff = np.asarray(lookup_coefficients, dtype=np.float32)
    idx = np.asarray(lookup_indices)

    # Compose per-output-channel filters on host (2.4 MFLOP - negligible).
    atoms = dictionary[idx]  # (Cout, S, Cin, K, K)
    weight = np.einsum("os,osckl->ockl", coeff, atoms)  # (Cout, Cin, 3, 3)
    # Winograd G @ w over ky -> lhsT [c, co, t, kx, o128]
    G = np.array([[1, 0, 0], [0.5, 0.5, 0.5], [0.5, -0.5, 0.5], [0, 0, 1]], np.float32)
    Gw = np.einsum("tk,ockx->ctxo", G, weight)  # (128, 4, 3, 256)
    Gw = Gw.reshape(CIN, NT, KK, 2, 128).transpose(0, 3, 1, 2, 4)
    wt_host = np.ascontiguousarray(Gw).astype(bf16)  # (128, 2, 4, 3, 128)

    # Pad, then Bt row transform: pairs p use padded rows 2p..2p+3.
    x_pad = np.zeros((B, CIN, HP, WP), dtype=np.float32)
    x_pad[:, :, 1 : H + 1, 1 : W + 1] = x
    d = [x_pad[:, :, k : k + 2 * (NPAIR - 1) + 1 : 2, :] for k in range(4)]
    XT = np.stack(
        [d[0] - d[2], d[1] + d[2], d[2] - d[1], d[1] - d[3]], axis=2
    )  # (B, C, 4t, 28p, 58)
    # regroup pairs into superblocks: (B, C, t, sb, p7, w) -> (B, C, sb, t, p7, w)
    XT = XT.reshape(B, CIN, NT, NSB, PSB, WP).transpose(0, 1, 3, 2, 4, 5)
    XT = np.ascontiguousarray(XT).astype(bf16)

    in_maps = []
    for c in range(N_CORES):
        xs_core = np.ascontiguousarray(
            XT[c * BPC : (c + 1) * BPC].transpose(1, 0, 2, 3, 4, 5)
        )  # (C, img, sb, t, p7, 58)
        in_maps.append({"xs": xs_core, "wt": wt_host})
    return in_maps


def _ensure_ntff_hook() -> bool:
    """Register the axon NTFF profile hook (missing antenv.axon_hooks shim).

    Only needed for trace=True runs; grading path (trace=False) never calls it.
    """
    import sys
    import types
    import contextlib
    import ctypes

    try:
        import antenv.axon_hooks as m  # noqa: F401
        if m.get_axon_ntff_profile_hook() is not None:
            return True
    except ImportError:
        m = types.ModuleType("antenv.axon_hooks")
        _h = {"hook": None}
        m.set_axon_ntff_profile_hook = lambda h: _h.__setitem__("hook", h)
        m.get_axon_ntff_profile_hook = lambda: _h["hook"]
        sys.modules["antenv.axon_hooks"] = m
        try:
            import antenv
            antenv.axon_hooks = m
        except ImportError:
            pass

    so_path = "/opt/axon/libaxon_pjrt.so"
    try:
        lib = ctypes.CDLL(so_path)
    except OSError:
        return False
    if not hasattr(lib, "axon_start_nrt_profile"):
        return False
    lib.axon_start_nrt_profile.argtypes = [
        ctypes.POINTER(ctypes.c_int64),
        ctypes.c_size_t,
    ]
    lib.axon_start_nrt_profile.restype = ctypes.c_int64
    lib.axon_stop_nrt_profile.argtypes = [ctypes.c_char_p]
    lib.axon_stop_nrt_profile.restype = ctypes.c_int64

    @contextlib.contextmanager
    def _hook(output_dir, device_ids):
        import jax

        jax.devices()
        if device_ids:
            ids = (ctypes.c_int64 * len(device_ids))(*device_ids)
            rc = lib.axon_start_nrt_profile(ids, len(device_ids))
        else:
            rc = lib.axon_start_nrt_profile(None, 0)
        if rc != 0:
            raise RuntimeError(f"axon_start_nrt_profile rc={rc}")
        try:
            yield
        finally:
            n = lib.axon_stop_nrt_profile(str(output_dir).encode())
            if n < 0:
                raise RuntimeError(f"axon_stop_nrt_profile rc={n}")
            print(f"profile: {n} file(s) written to {output_dir}", file=sys.stderr)

    m.set_axon_ntff_profile_hook(_hook)
    return True


def _run(inputs: dict, trace: bool = False):
    if trace:
        trace = _ensure_ntff_hook()
    nc = _get_program()
    in_maps = _prepare_inputs(**inputs)
    res = run_bass_kernel_spmd(nc, in_maps, list(range(N_CORES)), trace=trace)
    out = np.empty((B, COUT, H, W), dtype=np.float32)
    for c in range(N_CORES):
        # device layout: (p, img, co, eo, sb, p7, x)
        #   -> (img, co*128+p, 2*(sb*7+p7)+eo, x)
        arr = np.asarray(res.results[c]["out"]).reshape(128, BPC, 2, 2, NPAIR, W)
        out[c * BPC : (c + 1) * BPC] = (
            arr.transpose(1, 2, 0, 4, 3, 5)
            .reshape(BPC, COUT, H, W)
            .astype(np.float32)
        )
    return out, res


def kernel(**inputs) -> np.ndarray:
    out, _ = _run(inputs, trace=False)
    return out


# revision 18
# speedup vs baseline: 1.3673x; 1.0099x over previous
"""LookupConv2d kernel for Trainium2 (8 NeuronCores, data-parallel over batch).

Computation: weight[o] = sum_s coeff[o,s] * dictionary[idx[o,s]]  (tiny, host)
             out = conv2d(x, weight, stride 1, pad 1)             (device)

v2: Winograd F(2,3) along H in bf16 — 1.5x fewer PE MACs than direct conv.

  out[o, 2p+0, x] = m0 + m1 + m2          m_t[o,p,x] = sum_c sum_kx
  out[o, 2p+1, x] = m1 - m2 - m3                Gw[c,t,kx,o] * XT[c,t,p,x+kx]

  XT (host):  Bt rows of padded x:  [d0-d2, d1+d2, d2-d1, d1-d3],
              d_k = xpad[:, 2p+k, :]  -> [C, img, sb=4, t=4, p=7, 58] bf16
              (row-pair superblocks sb-major so each is one contiguous DMA)
  Gw (host):  G @ w over ky, G = [[1,0,0],[.5,.5,.5],[.5,-.5,.5],[0,0,1]]
              -> lhsT [C=128, co=2, t=4, kx=3, o=128] bf16

Device per core (4 images):
  - Per (img, co-half, superblock of 7 row-pairs): 12 matmuls (4t x 3kx)
    of N=392 accumulate m_t over kx into 4 PSUM banks (x2 sets for double
    buffering = all 8 banks).
  - ACT copies m0,m1,m2 PSUM->SBUF as bf16 (ACT is otherwise idle; keeps
    DVE off the 1x fp32 tensor_tensor path).
  - DVE (bf16 2x): t0=s0+s1; even=t0+s2; t1=s1-s2; odd=t1-m3(psum).
    Even/odd row planes stored separately; host interleaves (free).
  - All DMAs ride one HW queue in issue order, so issue them in consumption
    order: img0/sb0, co0 weights, img0/sb1-3, co1 weights, img1, imgs2+3.
  - PE warmup: dummy matmuls on a memset tile bridge the DMA wait so the
    HAM clock gate reaches 2.4 GHz before real matmuls start; absorber
    matmuls advance PE's clock past each input DMA so real matmuls carry
    at most one sync wait.
"""

import numpy as np
from contextlib import ExitStack

import concourse.bass as bass
import concourse.bacc as bacc
import concourse.tile as tile
from concourse import mybir
from concourse.bass_utils import run_bass_kernel_spmd

N_CORES = 8
B, CIN, H, W = 32, 128, 56, 56
COUT = 256
KK = 3
HP, WP = H + 2, W + 2  # padded 58, 58
BPC = B // N_CORES  # 4 images per core
NPAIR = H // 2  # 28 output row pairs
NT = 4  # winograd points
BF = mybir.dt.bfloat16
F32 = mybir.dt.float32

# 4 superblocks of 7 row-pairs each (uniform -> contiguous sb-major DMA)
NSB = 4
PSB = 7  # pairs per superblock; matmul N = 7*56 = 392
N_WARMUP = 108  # dummy matmuls (N=56, ~47ns cold) bridging preamble->data

_CACHE: dict = {}


def _build_program():
    nc = bacc.Bacc("TRN2", target_bir_lowering=False, debug=False)
    xs = nc.dram_tensor("xs", [CIN, BPC, NSB, NT, PSB, WP], BF, kind="ExternalInput")
    wt = nc.dram_tensor("wt", [CIN, 2, NT, KK, 128], BF, kind="ExternalInput")
    out = nc.dram_tensor(
        "out", [CIN, BPC, 2, 2, NSB, PSB, W], BF, kind="ExternalOutput"
    )

    with tile.TileContext(nc) as tc, ExitStack() as ctx:
        xpool = ctx.enter_context(tc.tile_pool(name="x", bufs=1))
        wpool = ctx.enter_context(tc.tile_pool(name="w", bufs=1))
        opool = ctx.enter_context(tc.tile_pool(name="o", bufs=1))
        ppool = ctx.enter_context(tc.tile_pool(name="p", bufs=1, space="PSUM"))
        spool = ctx.enter_context(tc.tile_pool(name="s", bufs=4))
        tpool = ctx.enter_context(tc.tile_pool(name="t", bufs=2))

        # 2 sets x 4 winograd points of PSUM accumulators = all 8 banks
        pt = [
            [
                ppool.tile([128, PSB, W], F32, name=f"ps{s}t{t}", tag=f"ps{s}t{t}")
                for t in range(NT)
            ]
            for s in range(2)
        ]
        scr = pt[1][0][:, 0, 0:2]  # absorber target (set B unused at start)

        wu = wpool.tile([128, 128], BF, tag="wu")
        nc.gpsimd.memset(wu[:], 0.0)
        for _ in range(N_WARMUP):
            nc.tensor.matmul(
                pt[1][0][:, 0, :], wu[:, 0:128], wu[:, 0:W], start=True, stop=True
            )

        def absorb(rhs2):
            nc.tensor.matmul(scr, wu[:, 0:128], rhs2, start=True, stop=True)

        # Input DMAs: each issuing engine triggers its own HW DMA ring, so
        # spreading issues across engines parallelizes the transfers.
        xt0 = xpool.tile([CIN, 1, NSB, NT, PSB, WP], BF, tag="x0")
        nc.sync.dma_start(xt0[:, :, 0], xs[:, 0:1, 0])
        w_all = wpool.tile([CIN, 2, NT, KK, 128], BF)
        nc.sync.dma_start(w_all[:, 0], wt[:, 0])
        nc.sync.dma_start(xt0[:, :, 1:NSB], xs[:, 0:1, 1:NSB])
        nc.sync.dma_start(w_all[:, 1], wt[:, 1])
        xt1 = xpool.tile([CIN, 1, NSB, NT, PSB, WP], BF, tag="x1")
        nc.sync.dma_start(xt1[:], xs[:, 1:2])
        xt23 = xpool.tile([CIN, 2, NSB, NT, PSB, WP], BF, tag="x23")
        nc.sync.dma_start(xt23[:], xs[:, 2:4])
        xv = [(xt0, 0), (xt1, 0), (xt23, 0), (xt23, 1)]

        absorb(xt0[:, 0, 0, 0, 0, 0:2])  # img0 sb0
        absorb(w_all[:, 0, 0, 0, 0:2])  # co0 weights

        sb = 0  # global superblock counter for PSUM set alternation
        ot01 = None
        for img in range(BPC):
            xt, j = xv[img]
            if img == 1:
                absorb(xt1[:, 0, 0, 0, 0, 0:2])
            elif img == 2:  # img3 shares img2's DMA; PE already observed it
                absorb(xt23[:, 0, 0, 0, 0, 0:2])
            if img == 0:
                ot01 = opool.tile([128, 2, 2, 2, NSB, PSB, W], BF, tag="o01")
                ot = ot01[:, 0]
            elif img == 1:
                ot = ot01[:, 1]
            else:
                ot = opool.tile([128, 2, 2, NSB, PSB, W], BF, tag=f"o{img}")
            for co in range(2):
                if img == 0 and co == 1:
                    absorb(w_all[:, 1, 0, 0, 0:2])  # co1 weights
                for sbi in range(NSB):
                    if img == 0 and co == 0 and sbi == 1:
                        absorb(xt0[:, 0, 1, 0, 0, 0:2])  # img0 sb1-3
                    s = sb % 2
                    sb += 1
                    # group order t1,t2,t3,t0: the only work left after the
                    # last (t0) matmul group is one DVE op -> minimal tail
                    for t in (1, 2, 3, 0):
                        for kx in range(KK):
                            nc.tensor.matmul(
                                pt[s][t][:],
                                w_all[:, co, t, kx, :],
                                xt[:, j, sbi, t, :, kx : kx + W],
                                start=(kx == 0),
                                stop=(kx == KK - 1),
                            )
                    # ACT: bf16 copies of m1,m2,m3 (PSUM -> SBUF)
                    sm = {}
                    for t in (1, 2, 3):
                        sm[t] = spool.tile([128, PSB, W], BF, name=f"sm{t}")
                        nc.scalar.copy(sm[t][:], pt[s][t][:])
                    # DVE: even = (m1+m2)+m0 ; odd = (m1-m2)-m3
                    t0 = tpool.tile([128, PSB, W], BF)
                    nc.vector.tensor_add(t0[:], sm[1][:], sm[2][:])
                    t1 = tpool.tile([128, PSB, W], BF)
                    nc.vector.tensor_sub(t1[:], sm[1][:], sm[2][:])
                    nc.vector.tensor_sub(ot[:, co, 1, sbi], t1[:], sm[3][:])
                    nc.vector.tensor_add(ot[:, co, 0, sbi], t0[:], pt[s][0][:])
                    if img == 3 and co == 1:
                        # stream the final image's last co-half out per
                        # superblock so the tail only waits on ~200KB
                        nc.sync.dma_start(out[:, 3, 1, :, sbi], ot[:, 1, :, sbi])
                if img == 3 and co == 0:
                    # flush img3/co0 while co1 computes
                    nc.sync.dma_start(out[:, 3, 0], ot[:, 0])
            if img == 1:
                nc.sync.dma_start(out[:, 0:2], ot01[:])
            elif img == 2:
                nc.sync.dma_start(out[:, 2], ot[:])
    nc.compile()
    return nc


def _get_program():
    if "nc" not in _CACHE:
        _CACHE["nc"] = _build_program()
    return _CACHE["nc"]


def _prepare_inputs(x, dictionary, lookup_coefficients, lookup_indices):
    import ml_dtypes

    bf16 = ml_dtypes.bfloat16
    x = np.asarray(x, dtype=np.float32)
    dictionary = np.asarray(dictionary, dtype=np.float32)
    coeff = np.asarray(lookup_coefficients, dtype=np.float32)
    idx = np.asarray(lookup_indices)

    # Compose per-output-channel filters on host (2.4 MFLOP - negligible).
    atoms = dictionary[idx]  # (Cout, S, Cin, K, K)
    weight = np.einsum("os,osckl->ockl", coeff, atoms)  # (Cout, Cin, 3, 3)
    # Winograd G @ w over ky -> lhsT [c, co, t, kx, o128]
    G = np.array([[1, 0, 0], [0.5, 0.5, 0.5], [0.5, -0.5, 0.5], [0, 0, 1]], np.float32)
    Gw = np.einsum("tk,ockx->ctxo", G, weight)  # (128, 4, 3, 256)
    Gw = Gw.reshape(CIN, NT, KK, 2, 128).transpose(0, 3, 1, 2, 4)
    wt_host = np.ascontiguousarray(Gw).astype(bf16)  # (128, 2, 4, 3, 128)

    # Pad, then Bt row transform: pairs p use padded rows 2p..2p+3.
    x_pad = np.zeros((B, CIN, HP, WP), dtype=np.float32)
    x_pad[:, :, 1 : H + 1, 1 : W + 1] = x
    d = [x_pad[:, :, k : k + 2 * (NPAIR - 1) + 1 : 2, :] for k in range(4)]
    XT = np.stack(
        [d[0] - d[2], d[1] + d[2], d[2] - d[1], d[1] - d[3]], axis=2
    )  # (B, C, 4t, 28p, 58)
    # regroup pairs into superblocks: (B, C, t, sb, p7, w) -> (B, C, sb, t, p7, w)
    XT = XT.reshape(B, CIN, NT, NSB, PSB, WP).transpose(0, 1, 3, 2, 4, 5)
    XT = np.ascontiguousarray(XT).astype(bf16)

    in_maps = []
    for c in range(N_CORES):
        xs_core = np.ascontiguousarray(
            XT[c * BPC : (c + 1) * BPC].transpose(1, 0, 2, 3, 4, 5)
        )  # (C, img, sb, t, p7, 58)
        in_maps.append({"xs": xs_core, "wt": wt_host})
    return in_maps


def _ensure_ntff_hook() -> bool:
    """Register the axon NTFF profile hook (missing antenv.axon_hooks shim).

    Only needed for trace=True runs; grading path (trace=False) never calls it.
    """
    import sys
    import types
    import contextlib
    import ctypes

    try:
        import antenv.axon_hooks as m  # noqa: F401
        if m.get_axon_ntff_profile_hook() is not None:
            return True
    except ImportError:
        m = types.ModuleType("antenv.axon_hooks")
        _h = {"hook": None}
        m.set_axon_ntff_profile_hook = lambda h: _h.__setitem__("hook", h)
        m.get_axon_ntff_profile_hook = lambda: _h["hook"]
        sys.modules["antenv.axon_hooks"] = m
        try:
            import antenv
            antenv.axon_hooks = m
        except ImportError:
            pass

    so_path = "/opt/axon/libaxon_pjrt.so"
    try:
        lib = ctypes.CDLL(so_path)
    except OSError:
        return False
    if not hasattr(lib, "axon_start_nrt_profile"):
        return False
    lib.axon_start_nrt_profile.argtypes = [
        ctypes.POINTER(ctypes.c_int64),
        ctypes.c_size_t,
    ]
    lib.axon_start_nrt_profile.restype = ctypes.c_int64
    lib.axon_stop_nrt_profile.argtypes = [ctypes.c_char_p]
    lib.axon_stop_nrt_profile.restype = ctypes.c_int64

    @contextlib.contextmanager
    def _hook(output_dir, device_ids):
        import jax

        jax.devices()
        if device_ids:
            ids = (ctypes.c_int64 * len(device_ids))(*device_ids)
            rc = lib.axon_start_nrt_profile(ids, len(device_ids))
        else:
            rc = lib.axon_start_nrt_profile(None, 0)
        if rc != 0:
            raise RuntimeError(f"axon_start_nrt_profile rc={rc}")
        try:
            yield
        finally:
            n = lib.axon_stop_nrt_profile(str(output_dir).encode())
            if n < 0:
                raise RuntimeError(f"axon_stop_nrt_profile rc={n}")
            print(f"profile: {n} file(s) written to {output_dir}", file=sys.stderr)

    m.set_axon_ntff_profile_hook(_hook)
    return True


def _run(inputs: dict, trace: bool = False):
    if trace:
        trace = _ensure_ntff_hook()
    nc = _get_program()
    in_maps = _prepare_inputs(**inputs)
    res = run_bass_kernel_spmd(nc, in_maps, list(range(N_CORES)), trace=trace)
    out = np.empty((B, COUT, H, W), dtype=np.float32)
    for c in range(N_CORES):
        # device layout: (p, img, co, eo, sb, p7, x)
        #   -> (img, co*128+p, 2*(sb*7+p7)+eo, x)
        arr = np.asarray(res.results[c]["out"]).reshape(128, BPC, 2, 2, NPAIR, W)
        out[c * BPC : (c + 1) * BPC] = (
            arr.transpose(1, 2, 0, 4, 3, 5)
            .reshape(BPC, COUT, H, W)
            .astype(np.float32)
        )
    return out, res


def kernel(**inputs) -> np.ndarray:
    out, _ = _run(inputs, trace=False)
    return out
